# revision 33
# baseline (speedup 1.0000x reference)
"""Trainium2 Bass kernel for 2-layer GRU encoder (nn_Encoder_75935021793540).

Problem: B=32, T=2048, D=256, U=512. Keras GRU v2 (reset_after=True) x 2 layers.
Returns (out2 [B,T,U], state1 [B,U], state2 [B,U]).

Sharding: data-parallel over batch across 8 cores (4 sequences/core), GRU
weights replicated; the time scan runs locally per core.

Per-core mapping (per time chunk of TC steps):
  - xproj (x @ kernel) computed as big-M matmuls into SBUF, rows = (t%32, b)
  - sequential scan: per step, gate matmuls accumulate in PSUM:
      pz = sel_t.T @ xp_z + sum_k hT_k @ rk_z   (same for r)
      phx = sel_t.T @ xp_h   (input projection of candidate, kept separate)
      phr = sum_k hT_k @ rk_h (+ b_rec_h)       (reset_after inner term)
    where sel_t is a 128x4 selector gathering step t's 4 batch rows out of
    the (t%32, b)-row xproj tile.
  - elementwise on ACT/DVE/GPSIMD; PE transposes h_new into lhsT layout; the
    transposed history doubles as layer-2's xproj input and the out staging.
"""

import numpy as np

import concourse.bass as bass
import concourse.mybir as mybir
import concourse.bacc as bacc
import concourse.tile as tile
from concourse.bass_utils import run_bass_kernel_spmd

F32 = mybir.dt.float32
F32R = mybir.dt.float32r


def _r(ap):
    """Bitcast an fp32 AP to float32r for full-rate PE streaming."""
    return ap.bitcast(F32R)
AF = mybir.ActivationFunctionType
ALU = mybir.AluOpType

NCORES = 8
BG, T_FULL, D, U = 32, 2048, 256, 512
BL = BG // NCORES  # 4 sequences per core
G3 = 3 * U

_nc_cache = {}


OPTS = {"notpos"}


def _build(T, TC, static_loop=False):
    """Build the SPMD Bass program for sequence length T, chunk TC.

    TC must be a multiple of 32 and divide T.
    """
    assert TC % 32 == 0 and T % TC == 0
    nchunk = T // TC
    nmt = TC // 32  # M-tiles (of 128 (t%32, b)-rows) per chunk
    MT = 32 * BL  # 128 rows per M-tile
    KD = D // 128  # k1 contraction chunks (2)
    KU = U // 128  # rk contraction chunks (4)

    nc = bacc.Bacc("TRN2", target_bir_lowering=False, debug=False)

    # ---- DRAM I/O (per-core shard; weights replicated) ----
    # xT: x pre-transposed on host to [KD, 128, T*BL] with free = t*BL + b
    xT_d = nc.dram_tensor("xT", [KD, 128, T * BL], F32R, kind="ExternalInput")
    h0_d = nc.dram_tensor("h0", [BL, U], F32, kind="ExternalInput")
    k1_d = nc.dram_tensor("k1r", [128, KD, G3], F32R, kind="ExternalInput")
    rk1_d = nc.dram_tensor("rk1r", [128, KU, G3], F32R, kind="ExternalInput")
    k2_d = nc.dram_tensor("k2r", [128, KU, G3], F32R, kind="ExternalInput")
    rk2_d = nc.dram_tensor("rk2r", [128, KU, G3], F32R, kind="ExternalInput")
    # bias rows: [z|r|h] input-side (b_in + b_rec for z,r; b_in for h)
    bias1_d = nc.dram_tensor("bias1", [1, G3], F32R, kind="ExternalInput")
    bias2_d = nc.dram_tensor("bias2", [1, G3], F32R, kind="ExternalInput")
    brh1_d = nc.dram_tensor("brh1", [1, U], F32R, kind="ExternalInput")
    brh2_d = nc.dram_tensor("brh2", [1, U], F32R, kind="ExternalInput")
    sel_d = nc.dram_tensor("sel", [128, 32, BL], F32R, kind="ExternalInput")
    ones_d = nc.dram_tensor("ones", [1, 128], F32R, kind="ExternalInput")
    eye4_d = nc.dram_tensor("eye4", [36, BL], F32, kind="ExternalInput")

    o2T_d = nc.dram_tensor("o2T", [128, KU, T * BL], F32R, kind="ExternalOutput")
    st1_d = nc.dram_tensor("st1", [BL, U], F32, kind="ExternalOutput")
    st2_d = nc.dram_tensor("st2", [BL, U], F32, kind="ExternalOutput")

    with tile.TileContext(nc) as tc:
        with (
            tc.tile_pool(name="w", bufs=1) as wp,  # persistent weights/state
            tc.tile_pool(name="io", bufs=2) as iop,  # chunk-rotating tiles
            tc.tile_pool(name="ew", bufs=1) as ewp,
            tc.tile_pool(name="xh", bufs=2) as xhp,  # small elementwise tiles
            tc.tile_pool(name="ps", bufs=1, space="PSUM") as psp,
            tc.tile_pool(name="pt", bufs=2, space="PSUM") as ptp,
        ):
            # ---- persistent SBUF tensors ----
            k1_s = wp.tile([128, KD, G3], F32R, tag="k1")
            rk1_s = wp.tile([128, KU, G3], F32R, tag="rk1")
            k2_s = wp.tile([128, KU, G3], F32R, tag="k2")
            rk2_s = wp.tile([128, KU, G3], F32R, tag="rk2")
            bias1_s = wp.tile([1, G3], F32R, tag="b1")
            bias2_s = wp.tile([1, G3], F32R, tag="b2")
            brh1_s = wp.tile([1, U], F32R, tag="brh1")
            brh2_s = wp.tile([1, U], F32R, tag="brh2")
            ones1_s = wp.tile([1, 128], F32R, tag="ones1")
            sel_s = wp.tile([128, 32, BL], F32R, tag="sel")
            eye4_s = wp.tile([36, BL], F32, tag="eye4")
            h1m = wp.tile([BL, U], F32, tag="h1m")  # master h, layer 1
            h2m = wp.tile([36, U], F32, tag="h2m")
            # transposed h history; [:, k, mt, tl*BL + b] = h_{mt*32+tl}^T
            o1T = wp.tile([128, KU, nmt, MT], F32R, tag="o1T")
            o2T = wp.tile([128, KU, nmt, MT], F32R, tag="o2T")
            # xproj chunk buffers, rows (t%32, b) on partitions
            xp1 = wp.tile([128, nmt, 3, U], F32R, tag="xp1")
            xp2 = wp.tile([128, nmt, 3, U], F32R, tag="xp2")

            nc.sync.dma_start(k1_s[:], k1_d[:])
            nc.sync.dma_start(rk1_s[:], rk1_d[:])
            nc.sync.dma_start(k2_s[:], k2_d[:])
            nc.sync.dma_start(rk2_s[:], rk2_d[:])
            nc.sync.dma_start(bias1_s[:], bias1_d[:])
            nc.sync.dma_start(bias2_s[:], bias2_d[:])
            nc.sync.dma_start(brh1_s[:], brh1_d[:])
            nc.sync.dma_start(brh2_s[:], brh2_d[:])
            nc.sync.dma_start(sel_s[:], sel_d[:])
            nc.sync.dma_start(eye4_s[:], eye4_d[:])
            nc.sync.dma_start(h1m[:], h0_d[:])
            nc.sync.dma_start(ones1_s[:], ones_d[:])

            # seed transposed h0 into the last time slot of both histories
            tp0 = ptp.tile([128, KU * BL], F32, tag="tp")
            for k in range(KU):
                nc.tensor.transpose(
                    tp0[:, k * BL:(k + 1) * BL],
                    h1m[:, k * 128:(k + 1) * 128],
                    eye4_s[0:BL, :],
                )
            nc.vector.tensor_copy(o1T[:, :, nmt - 1, MT - BL:MT], tp0[:])
            nc.vector.tensor_copy(o2T[:, :, nmt - 1, MT - BL:MT], tp0[:])

            def xproj(xp, lhsT_of, kw_s, nk, bias_s, only_mt=None):
                """xp[:, mt, g, :] = lhsT(mt).T @ kw[:, :, gU:(g+1)U] + bias."""
                mts = range(nmt) if only_mt is None else [only_mt]
                for mt in mts:
                    for g in range(3):
                        pxp = ptp.tile([128, U], F32, tag="tp")
                        nc.tensor.matmul(
                            pxp[:],
                            (ones1_s[:]),
                            (bias_s[:, g * U:(g + 1) * U]),
                            start=True,
                            stop=False,
                        )
                        for k in range(nk):
                            nc.tensor.matmul(
                                pxp[:],
                                (lhsT_of(k, mt)),
                                (kw_s[:, k, g * U:(g + 1) * U]),
                                start=False,
                                stop=(k == nk - 1),
                            )
                        nc.vector.tensor_copy(xp[:, mt, g, :], pxp[:])

            def scan_step_gen(xp, rk_s, hm, oT, t, poff, lid):
                """One GRU step; lanes poff..poff+BL."""
                tpos = (0, poff) if (poff and "notpos" not in OPTS) else None
                pl, ph = poff, poff + BL
                mt, tl = t // 32, t % 32
                tp_, tlp = ((t - 1) % TC) // 32, ((t - 1) % TC) % 32
                selt = sel_s[:, tl, :]
                hT = [oT[:, k, tp_, tlp * BL:(tlp + 1) * BL]
                      for k in range(KU)]

                pz = psp.tile([ph, U], F32, tag=f"pz{lid}")
                pr = psp.tile([ph, U], F32, tag=f"pr{lid}")
                phr = psp.tile([ph, U], F32, tag=f"phr{lid}")
                xh = xhp.tile([ph, U], F32, tag=f"xh{lid}")
                nc.sync.dma_start(
                    xh[pl:ph, :],
                    xp[tl * BL:(tl + 1) * BL, mt, 2, :].bitcast(F32),
                )
                nc.tensor.matmul(pr[pl:ph, :], selt, xp[:, mt, 1, :],
                                 start=True, stop=False, tile_position=tpos)
                for k in range(KU):
                    nc.tensor.matmul(pr[pl:ph, :], hT[k], rk_s[:, k, U:2 * U],
                                     start=False, stop=(k == KU - 1),
                                     tile_position=tpos)
                yield
                for k in range(KU):
                    nc.tensor.matmul(phr[pl:ph, :], hT[k], rk_s[:, k, 2 * U:G3],
                                     start=(k == 0), stop=(k == KU - 1),
                                     tile_position=tpos)
                yield
                nc.tensor.matmul(pz[pl:ph, :], selt, xp[:, mt, 0, :],
                                 start=True, stop=False, tile_position=tpos)
                for k in range(KU):
                    nc.tensor.matmul(pz[pl:ph, :], hT[k], rk_s[:, k, 0:U],
                                     start=False, stop=(k == KU - 1),
                                     tile_position=tpos)
                yield

                r = ewp.tile([ph, U], F32, tag=f"r{lid}")
                zc = ewp.tile([ph, U], F32, tag=f"zc{lid}")
                v1 = ewp.tile([ph, U], F32, tag=f"v1{lid}")
                v2 = ewp.tile([ph, U], F32, tag=f"v2{lid}")
                d = v1
                e = v2
                hh = ewp.tile([ph, U], F32, tag=f"hh{lid}")

                nc.scalar.activation(r[pl:ph, :], pr[pl:ph, :], AF.Sigmoid)
                yield
                nc.vector.tensor_tensor(v1[pl:ph, :], r[pl:ph, :],
                                        phr[pl:ph, :], ALU.mult)
                yield
                nc.vector.tensor_tensor(v2[pl:ph, :], v1[pl:ph, :],
                                        xh[pl:ph, :], ALU.add)
                yield
                nc.scalar.activation(hh[pl:ph, :], v2[pl:ph, :], AF.Tanh)
                # zc = 1 - z = sigmoid(-pz); h_new = h - zc*(h - hh)
                nc.scalar.activation(zc[pl:ph, :], pz[pl:ph, :], AF.Sigmoid,
                                     scale=-1.0)
                yield
                nc.vector.tensor_tensor(d[pl:ph, :], hm[pl:ph, :],
                                        hh[pl:ph, :], ALU.subtract)
                yield
                nc.vector.tensor_tensor(e[pl:ph, :], zc[pl:ph, :],
                                        d[pl:ph, :], ALU.mult)
                yield
                nc.vector.tensor_tensor(hm[pl:ph, :], hm[pl:ph, :],
                                        e[pl:ph, :], ALU.subtract)
                yield
                tp = ptp.tile([128, KU * BL], F32, tag="tp")
                for k in range(KU):
                    nc.tensor.transpose(
                        tp[:, k * BL:(k + 1) * BL],
                        hm[pl:ph, k * 128:(k + 1) * 128],
                        eye4_s[pl:ph, :],
                    )
                nc.vector.tensor_copy(
                    oT[:, :, mt, tl * BL:(tl + 1) * BL], tp[:]
                )

            L2OFF = 0 if "notpos" in OPTS else 32
            nc.sync.dma_start(h2m[L2OFF:L2OFF + BL, :], h0_d[:])

            def _drain(g):
                for _ in g:
                    pass

            def weave_blocks(mt1, mt2):
                """Emit L1 block mt1 and L2 block mt2 instruction-zipped."""
                for i in range(32):
                    g1 = g2 = None
                    if mt1 is not None:
                        g1 = scan_step_gen(xp1, rk1_s, h1m, o1T,
                                           mt1 * 32 + i, 0, 1)
                    if mt2 is not None and "l1only" not in OPTS:
                        g2 = scan_step_gen(xp2, rk2_s, h2m, o2T,
                                           mt2 * 32 + i, L2OFF, 2)
                    if g1 is None:
                        _drain(g2)
                    elif g2 is None:
                        _drain(g1)
                    elif "zip" in OPTS:
                        a_live = b_live = True
                        while a_live or b_live:
                            if a_live:
                                try:
                                    next(g1)
                                except StopIteration:
                                    a_live = False
                            if b_live:
                                try:
                                    next(g2)
                                except StopIteration:
                                    b_live = False
                    else:
                        _drain(g1)
                        _drain(g2)

            def chunk_body(ci):
                xt0 = iop.tile([128, nmt, MT], F32R, tag="xt0")
                xt1 = iop.tile([128, nmt, MT], F32R, tag="xt1")
                nc.sync.dma_start(xt0[:], xT_d[0, :, bass.ts(ci, TC * BL)])
                nc.sync.dma_start(xt1[:], xT_d[1, :, bass.ts(ci, TC * BL)])
                xts = [xt0, xt1]

                xproj(
                    xp1,
                    lambda k, mt: xts[k][:, mt, :],
                    k1_s, KD, bias1_s,
                )
                # software pipeline, 1-block skew: L2 trails L1 by one
                # 32-step block; L2's last block of chunk ci runs woven
                # with L1's first block of chunk ci+1 (o2T slot nmt-1 of
                # the previous chunk is still valid since o2T DMA-out
                # completes before the next overwrite).
                weave_blocks(0, None)
                for mt in range(1, nmt):
                    xproj(xp2, lambda k, _mt: o1T[:, k, _mt, :],
                          k2_s, KU, bias2_s, only_mt=mt - 1)
                    weave_blocks(mt, mt - 1)
                xproj(xp2, lambda k, _mt: o1T[:, k, _mt, :],
                      k2_s, KU, bias2_s, only_mt=nmt - 1)
                weave_blocks(None, nmt - 1)
                nc.sync.dma_start(
                    o2T_d[:, :, bass.ts(ci, TC * BL)],
                    o2T[:].rearrange("p k m j -> p (k m j)"),
                )

            if static_loop:
                for ci in range(nchunk):
                    chunk_body(ci)
            else:
                with tc.For_i(0, nchunk, 1) as ci:
                    chunk_body(ci)

            nc.sync.dma_start(st1_d[:], h1m[:])
            nc.sync.dma_start(st2_d[:], h2m[L2OFF:L2OFF + BL, :])

    nc.compile()
    return nc


def _prep_shared(k1, rk1, b1, k2, rk2, b2):
    """Host-side weight re-layouts (replicated to all cores)."""
    def wsplit(w, nk):  # [K, G3] -> [128, nk, G3]
        return np.ascontiguousarray(
            w.reshape(nk, 128, -1).transpose(1, 0, 2)
        ).astype(np.float32)

    sel = np.zeros((128, 32, BL), np.float32)
    for c in range(BL):
        for tl in range(32):
            sel[tl * BL + c, tl, c] = 1.0

    def biasrow(b):
        bb = b[0].copy()
        bb[: 2 * U] += b[1][: 2 * U]  # b_rec folds in for z, r only
        return bb.reshape(1, -1).astype(np.float32)

    shared = {
        "k1r": wsplit(k1, D // 128),
        "rk1r": wsplit(rk1, U // 128),
        "k2r": wsplit(k2, U // 128),
        "rk2r": wsplit(rk2, U // 128),
        "bias1": biasrow(b1),
        "bias2": biasrow(b2),
        "brh1": np.ascontiguousarray(b1[1][2 * U:]).reshape(1, U).astype(np.float32),
        "brh2": np.ascontiguousarray(b2[1][2 * U:]).reshape(1, U).astype(np.float32),
        "sel": sel,
        "ones": np.ones((1, 128), np.float32),
        "eye4": np.concatenate(
            [np.eye(BL), np.zeros((28, BL)), np.eye(BL)], axis=0
        ).astype(np.float32),
    }
    brh_nz = bool(np.any(b1[1][2 * U:]) or np.any(b2[1][2 * U:]))
    assert not brh_nz, "nonzero recurrent candidate bias not enabled in build"
    return shared


def kernel(x, hidden, k1, rk1, b1, k2, rk2, b2, _T=None, _TC=128, _trace=False):
    x = np.asarray(x, np.float32)
    hidden = np.asarray(hidden, np.float32)
    T = x.shape[1] if _T is None else _T
    key = (T, _TC)
    if key not in _nc_cache:
        _nc_cache[key] = _build(T, _TC)
    nc = _nc_cache[key]

    shared = _prep_shared(*[np.asarray(a, np.float32)
                            for a in (k1, rk1, b1, k2, rk2, b2)])

    KD, KU = D // 128, U // 128
    in_maps = []
    for c in range(NCORES):
        xs = x[c * BL:(c + 1) * BL, :T]  # [BL, T, D]
        # -> [KD, 128, T*BL] with free index t*BL + b
        xT = np.ascontiguousarray(
            xs.transpose(2, 1, 0).reshape(KD, 128, T * BL)
        )
        m = dict(shared)
        m["xT"] = xT
        m["h0"] = np.ascontiguousarray(hidden[c * BL:(c + 1) * BL])
        in_maps.append(m)

    res = run_bass_kernel_spmd(
        nc, in_maps, core_ids=list(range(NCORES)), trace=_trace
    )

    out2 = np.empty((BG, T, U), np.float32)
    st1 = np.empty((BG, U), np.float32)
    st2 = np.empty((BG, U), np.float32)
    for c in range(NCORES):
        r = res.results[c]
        # o2T [128, KU, T*BL] -> [BL, T, KU*128]
        o = r["o2T"].reshape(128, KU, T, BL)
        out2[c * BL:(c + 1) * BL] = o.transpose(3, 2, 1, 0).reshape(BL, T, U)
        st1[c * BL:(c + 1) * BL] = r["st1"]
        st2[c * BL:(c + 1) * BL] = r["st2"]
    kernel.last_results = res
    return out2, st1, st2
